# revision 47
# baseline (speedup 1.0000x reference)




"""MinGRU Trainium2 kernel (nn_MinGRU_60421599920446).

Math (per batch row), with z = sigmoid(x@wz^T + bz), vh = x@wh^T:
    h_t = (1-z_t) h_{t-1} + z_t (vh_t + bh)
Substituting c_t = h_t - bh eliminates the bh bias from the device:
    c_t = (1-z_t) c_{t-1} + z_t vh_t,   c_0 = h0 - bh
The host adds bh back (and transposes) when assembling the output.

Strategy: data-parallel over batch, 1 row per NeuronCore (8 cores).
The host pre-transposes x to xT [D, S] in bf16, so the device does no
PE transposes and works natively in the scan layout [H partitions, S free].

The whole recurrence runs as ONE hand-written custom DVE op
(MINGRU_SCAN_ANT) that streams z (SBUF) and vh (PSUM fp32) directly:
    phase A (consumes z_t, v_t): u = 1-z; m = z*v; state *= u
    phase B (no consume):        state += m; emit state
Two uOp phases per timestep -> 2 cycles/step, same rate as the stock
tensor_tensor_scan, but with no a/b tensors, no PSUM->SBUF copies and
no Pool work at all.

Per 1024-step chunk (two H-halves m=0,1):
    sync DMA : xT[k] [128,1024] bf16 in
    PE       : vz[m], vh[m] [128,1024] fp32 PSUM   (16 matmuls)
    ACT      : z[m] = Sigmoid(vz[m]+bz) bf16
    DVE      : c[m] = MINGRU_SCAN_ANT(z[m], vh[m], carry) bf16
    sync DMA : c[m] -> cT [256, S] bf16 out
"""

import numpy as np
from contextlib import ExitStack

B, S, D, H = 8, 8192, 256, 256
N_CORES = 8

_CACHE = {}

OP_NAME = "MINGRU_SCAN_ANT"


def _register_mingru_op():
    """Register the hand-written fused scan as a custom DVE op (idempotent)."""
    import concourse.dve_ops as dve_ops
    for o in dve_ops.OPS:
        if o.name == OP_NAME:
            return o

    from concourse.dve_spec import Spec, Src0, Src1, C0
    from concourse.dve_uop import (
        DveOpSpec, UopConfig, UopDpConfig, AluOp, AluInp, InpSel, OutSel,
        OutPath, Trigger, DelayInp, ENABLE, N_STAGES,
    )

    def ref(in0, in1, c0, c1, c2):
        z = np.asarray(in0, np.float32)
        v = np.asarray(in1, np.float32)
        if z.ndim == 3:   # stride-0 repeat dim at the call site
            z = z[:, :, 0]
        if v.ndim == 3:
            v = v[:, :, 0]
        s = np.array(np.broadcast_to(c0, (z.shape[0], 1))[:, 0], np.float32)
        out = np.empty_like(z)
        for t in range(z.shape[1]):
            s = (1.0 - z[:, t]) * s + z[:, t] * v[:, t]
            out[:, t] = s
        return out

    spec = Spec(body=Src0 * Src1 + C0, reference=ref)  # body is a stand-in;
    # compile() below supplies the hand-written uOp program instead of lower().

    def build_uops(ver):
        n_stages = N_STAGES[ver]

        def mk():
            u = UopConfig()
            u.datapath_config = [UopDpConfig() for _ in range(n_stages)]
            return u

        # uop 0 -- seed: plant the carry (CONST_0, per-partition) in blk2's
        # out-flop. Consumes nothing, runs for exactly one slot.
        seed = mk()
        seed.enable_input(InpSel.SRC_0, 1)
        seed.enable_input(InpSel.SRC_1, 2)
        seed.enable_input(InpSel.CONST_0, 3)
        seed.datapath_config[0].pass_through_alu().pass_through_delay(2)
        seed.datapath_config[1].pass_through_alu().pass_through_delay(2)
        seed.datapath_config[2].enable_alu(AluOp.BYPASS, AluInp.PREV_DELAY_2)
        for k in range(3, n_stages):
            seed.datapath_config[k].pass_through_alu()
        seed.repeat_count = 1
        seed.trigger = (Trigger.COUNT, Trigger.NONE, Trigger.NONE)
        seed.next_uop = (1, 0, 0)

        # uop 1 -- phase A: consume (z_t, v_t). u = 1-z (blk0); m = z*v
        # (blk1, left in blk1's out-flop for phase B); state *= u (blk2,
        # via blk2's own CURR_ALU_OUT feedback).
        A = mk()
        A.enable_input(InpSel.SRC_0, 1)       # chain0 = z
        A.enable_input(InpSel.SRC_1, 2)       # chain1 = v
        A.enable_input(InpSel.ONE_F32, 3)     # chain2 = 1.0
        A.require_inp0 = ENABLE
        A.require_inp1 = ENABLE
        A.datapath_config[0].enable_alu(
            AluOp.SUBTRACT, AluInp.PREV_DELAY_2, AluInp.PREV_DELAY_0
        ).pass_through_delay(0, 1)
        A.datapath_config[1].enable_alu(
            AluOp.MULTIPLY, AluInp.PREV_DELAY_0, AluInp.PREV_DELAY_1
        ).enable_delay_from_src(DelayInp.PREV_ALU_OUT, 2)   # chain2 <- u
        A.datapath_config[2].enable_alu(
            AluOp.MULTIPLY, AluInp.CURR_ALU_OUT, AluInp.PREV_DELAY_2
        )
        for k in range(3, n_stages):
            A.datapath_config[k].pass_through_alu()
        A.repeat_count = 1
        # COUNT first: after consuming an element ALWAYS hand off to phase B
        # (even for the final element — exiting on SRC_TENSOR_DONE here would
        # skip the last emit and hang the dst-write count). SRC_TENSOR_DONE
        # only fires when A re-enters with the stream already exhausted.
        A.trigger = (Trigger.COUNT, Trigger.SRC_TENSOR_DONE, Trigger.NONE)
        A.next_uop = (2, 0, 0)

        # uop 2 -- phase B: no consume. blk1 re-presents m; blk2 adds it to
        # the state (CURR_ALU_OUT = u*state written by A one cycle earlier)
        # and the result rides to blk7 and is written out.
        # Each stream element is delivered TWICE (stride-0 inner AP dim at
        # the call site), so phase B is a real consuming element too — its
        # inputs are the same (z_t, v_t) again, ignored by the datapath.
        Bu = mk()
        Bu.enable_input(InpSel.SRC_0, 1)
        Bu.enable_input(InpSel.SRC_1, 2)
        Bu.require_inp0 = ENABLE
        Bu.require_inp1 = ENABLE
        Bu.datapath_config[1].enable_alu(AluOp.BYPASS, AluInp.CURR_ALU_OUT)
        Bu.datapath_config[2].enable_alu(
            AluOp.ADD, AluInp.CURR_ALU_OUT, AluInp.PREV_ALU_OUT
        )
        for k in range(3, n_stages):
            Bu.datapath_config[k].pass_through_alu()
        Bu.enable_output(OutSel.ALU_OUT, OutPath.WR0_LO)
        Bu.repeat_count = 1
        # B consumes the final stream element (index 2N-1): SRC_TENSOR_DONE
        # must fire HERE so the sequencer reaches IDLE and the next custom
        # instruction can dispatch (exiting only via a stalled A leaves the
        # FSM parked and hangs the following custom-DVE instruction).
        Bu.trigger = (Trigger.SRC_TENSOR_DONE, Trigger.COUNT, Trigger.NONE)
        Bu.next_uop = (0, 1, 0)

        for u in (seed, A, Bu):
            u.validate(ver)
        return [seed, A, Bu]

    class HandDveOp(dve_ops.DveOp):
        def compile(self, ver):
            key = (self.name, ver)
            cache = dve_ops._COMPILE_CACHE
            if key not in cache:
                cache[key] = DveOpSpec(
                    name=self.name,
                    opcode=dve_ops.get_dve_sub_opcode(self.name),
                    uops=build_uops(ver),
                    rd1_en=True,
                )
            return cache[key]

    op = HandDveOp(name=OP_NAME, spec=spec, subdim=False, uops_sha={})
    dve_ops.OPS.append(op)
    dve_ops.CUSTOM_DVE_SPECS[OP_NAME] = spec
    dve_ops._SUB_OPCODE_FOR_NAME[OP_NAME] = (
        dve_ops._CUSTOM_DVE_ROW_BASE + len(dve_ops.OPS) - 1
    )
    assert dve_ops._SUB_OPCODE_FOR_NAME[OP_NAME] < 0x20
    return op


OP2_NAME = "MINGRU_SCAN2_ANT"


def _register_mingru2_op():
    """Interleaved-halves fused scan: ONE instruction processes both H-halves
    at 1 timestep/cycle aggregate (2x the stock tensor_tensor_scan rate).

    Stream slots alternate halves (even slot = half0, odd = half1); the
    call site's 3D APs deliver (t, m)-interleaved elements from contiguous
    per-half tiles. Each slot consumes (z, v) of its half and performs the
    full update c = (1-z)c + z*v. Per-half state lives in a stage A-flop
    (half0: s4.A, half1: s5.A), read one stage earlier via NEXT_ALU_OUT_A:
    a 2-cycle feedback loop matching each half's 2-slot cadence.

    Carries enter THROUGH THE STREAM: the first (t=0) pair per half must be
    (z=1, v=carry), which sets state=carry exactly; its echoed output is
    sliced away by the caller. No seed uop, no scalar operands."""
    import concourse.dve_ops as dve_ops
    for o in dve_ops.OPS:
        if o.name == OP2_NAME:
            return o

    from concourse.dve_spec import Spec, Src0, Src1
    from concourse.dve_uop import (
        DveOpSpec, UopConfig, UopDpConfig, AluOp, AluInp, InpSel, OutSel,
        OutPath, Trigger, DelayInp, ENABLE, N_STAGES,
    )

    def ref(in0, in1, c0, c1, c2):
        z = np.asarray(in0, np.float32)
        v = np.asarray(in1, np.float32)
        flat = z.ndim == 2
        if flat:   # [P, 2T] memory-interleaved stream
            z = z.reshape(z.shape[0], -1, 2)
            v = v.reshape(v.shape[0], -1, 2)
        P, T, _ = z.shape
        st = [np.zeros(P, np.float32), np.zeros(P, np.float32)]
        out = np.empty_like(z)
        for t in range(T):
            for m in range(2):
                st[m] = (1.0 - z[:, t, m]) * st[m] + z[:, t, m] * v[:, t, m]
                out[:, t, m] = st[m]
        return out.reshape(P, -1) if flat else out

    spec = Spec(body=Src0 * Src1, reference=ref)  # stand-in body

    def build_uops(ver):
        n_stages = N_STAGES[ver]

        def steady(mult_stage, add_stage):
            U = UopConfig()
            U.datapath_config = [UopDpConfig() for _ in range(n_stages)]
            U.enable_input(InpSel.SRC_0, 1)       # chain0 = z
            U.enable_input(InpSel.SRC_1, 2)       # chain1 = v
            U.enable_input(InpSel.ONE_F32, 3)     # chain2 = 1.0
            U.require_inp0 = ENABLE
            U.require_inp1 = ENABLE
            # s0: u = 1 - z
            U.datapath_config[0].enable_alu(
                AluOp.SUBTRACT, AluInp.PREV_DELAY_2, AluInp.PREV_DELAY_0
            ).pass_through_delay(0, 1)
            # s1: m = z*v ; chain2 <- u
            U.datapath_config[1].enable_alu(
                AluOp.MULTIPLY, AluInp.PREV_DELAY_0, AluInp.PREV_DELAY_1
            ).enable_delay_from_src(DelayInp.PREV_ALU_OUT, 2)
            # s2: chain3 <- m ; carry chain2 (u)
            U.datapath_config[2].pass_through_alu().pass_through_delay(2)
            U.datapath_config[2].enable_delay_from_src(DelayInp.PREV_ALU_OUT, 3)
            for k in range(3, mult_stage):
                U.datapath_config[k].pass_through_alu().pass_through_delay(2, 3)
            # mult_stage: p = state * u  (state = A-flop of mult_stage+1)
            U.datapath_config[mult_stage].enable_alu(
                AluOp.MULTIPLY, AluInp.NEXT_ALU_OUT_A, AluInp.PREV_DELAY_2
            ).pass_through_delay(3)
            # add_stage: c' = p + m ; state <- c' (own A-flop)
            blk = U.datapath_config[add_stage]
            blk.enable_alu(AluOp.ADD, AluInp.PREV_ALU_OUT, AluInp.PREV_DELAY_3)
            blk.alu_out_a_enable = ENABLE
            for k in range(add_stage + 1, n_stages):
                U.datapath_config[k].pass_through_alu()
            U.enable_output(OutSel.ALU_OUT, OutPath.WR0_LO)
            U.repeat_count = 1
            return U

        # uops[0] cannot be looped back to, so the H0 config appears twice:
        # entry (index 0) and loop body (index 2).
        H0a = steady(3, 4)
        H0a.trigger = (Trigger.COUNT, Trigger.SRC_TENSOR_DONE, Trigger.NONE)
        H0a.next_uop = (1, 0, 0)
        H1 = steady(4, 5)
        H1.trigger = (Trigger.SRC_TENSOR_DONE, Trigger.COUNT, Trigger.NONE)
        H1.next_uop = (0, 2, 0)
        H0b = steady(3, 4)
        H0b.trigger = (Trigger.COUNT, Trigger.SRC_TENSOR_DONE, Trigger.NONE)
        H0b.next_uop = (1, 0, 0)

        for u in (H0a, H1, H0b):
            u.validate(ver)
        return [H0a, H1, H0b]

    class HandDveOp(dve_ops.DveOp):
        def compile(self, ver):
            key = (self.name, ver)
            cache = dve_ops._COMPILE_CACHE
            if key not in cache:
                cache[key] = DveOpSpec(
                    name=self.name,
                    opcode=dve_ops.get_dve_sub_opcode(self.name),
                    uops=build_uops(ver),
                    rd1_en=True,
                )
            return cache[key]

    op = HandDveOp(name=OP2_NAME, spec=spec, subdim=False, uops_sha={})
    dve_ops.OPS.append(op)
    dve_ops.CUSTOM_DVE_SPECS[OP2_NAME] = spec
    dve_ops._SUB_OPCODE_FOR_NAME[OP2_NAME] = (
        dve_ops._CUSTOM_DVE_ROW_BASE + len(dve_ops.OPS) - 1
    )
    assert dve_ops._SUB_OPCODE_FOR_NAME[OP2_NAME] < 0x20
    return op


def _build(seq_len, chunk):
    """Build + compile the single-core SPMD Bass program."""
    import concourse.bacc as bacc
    import concourse.tile as tile
    import concourse.mybir as mybir

    dt = mybir.dt
    f32 = dt.float32
    bf16 = dt.bfloat16
    AF = mybir.ActivationFunctionType

    mingru_op = _register_mingru_op()

    assert chunk % 512 == 0 and seq_len % chunk == 0
    if seq_len // chunk >= 4:
        # taper: small first/last chunks shorten pipeline fill (first scan
        # starts sooner) and drain (smaller final scans + output DMA)
        csizes = ([chunk // 2] + [chunk] * (seq_len // chunk - 1)
                  + [chunk // 2])
    else:
        csizes = [chunk] * (seq_len // chunk)
    assert sum(csizes) == seq_len

    nc = bacc.Bacc("TRN2", target_bir_lowering=False, debug=False)

    xT_d = nc.dram_tensor("xT", [D, seq_len], bf16, kind="ExternalInput").ap()
    # packed weights: [wz k0 | wz k1 | wh k0 | wh k1], each [128, H]
    wall_d = nc.dram_tensor("wall", [128, 4 * H], bf16, kind="ExternalInput").ap()
    # packed per-partition columns: [c0_m0, bz_m0, c0_m1, bz_m1]
    cols_d = nc.dram_tensor("cols", [128, 4], f32, kind="ExternalInput").ap()
    out_d = nc.dram_tensor("out", [H, seq_len], f32, kind="ExternalOutput").ap()

    with tile.TileContext(nc) as tc, ExitStack() as ctx:
        const = ctx.enter_context(tc.tile_pool(name="const", bufs=1))
        xin = ctx.enter_context(tc.tile_pool(name="xin", bufs=6))
        zp = ctx.enter_context(tc.tile_pool(name="z", bufs=6))
        vbp = ctx.enter_context(tc.tile_pool(name="vb", bufs=6))
        cp = ctx.enter_context(tc.tile_pool(name="c", bufs=4))
        vzp = ctx.enter_context(tc.tile_pool(name="vz", bufs=2, space="PSUM"))
        vhp = ctx.enter_context(tc.tile_pool(name="vh", bufs=2, space="PSUM"))

        cols = const.tile([128, 4], f32, tag="cols", name="cols")
        nc.sync.dma_start(cols[:], cols_d[:, :])
        wall = const.tile([128, 4 * H], bf16, tag="wall", name="wall")
        nc.sync.dma_start(wall[:], wall_d[:, :])
        # lhsT slice for matrix j (0=z, 1=h), k-half k, output half m
        def wsl(j, k, m):
            o = j * 2 * H + k * H + m * 128
            return wall[:, o:o + 128]

        c_hist = {}

        c_start = 0
        for c, cs in enumerate(csizes):
            mw = min(cs, 512)
            n512 = cs // mw
            xt = xin.tile([128, 2 * cs], bf16, tag="xt", name="xt")
            dma_eng = nc.sync
            dma_eng.dma_start(
                xt[:].rearrange("p (k s) -> p k s", k=2),
                xT_d.rearrange("(k p) s -> p k s", k=2)[
                    :, :, c_start:c_start + cs])

            for m in range(2):
                vz = vzp.tile([128, cs], f32, tag="vz", name=f"vz{m}")
                for k in range(2):
                    for s2 in range(n512):
                        nc.tensor.matmul(
                            vz[:, s2 * mw:(s2 + 1) * mw],
                            wsl(0, k, m),
                            xt[:, k * cs + s2 * mw:
                               k * cs + (s2 + 1) * mw],
                            start=(k == 0), stop=(k == 1),
                        )
                vh = vhp.tile([128, cs], f32, tag="vh", name=f"vh{m}")
                for k in range(2):
                    for s2 in range(n512):
                        nc.tensor.matmul(
                            vh[:, s2 * mw:(s2 + 1) * mw],
                            wsl(1, k, m),
                            xt[:, k * cs + s2 * mw:
                               k * cs + (s2 + 1) * mw],
                            start=(k == 0), stop=(k == 1),
                        )

                z = zp.tile([128, cs], bf16, tag="z", name=f"z{m}")
                nc.scalar.activation(z[:], vz[:], AF.Sigmoid,
                                     bias=cols[:, 2 * m + 1:2 * m + 2],
                                     scale=1.0)
                # custom DVE ops cannot read PSUM -> stage vh in SBUF (bf16)
                vb = vbp.tile([128, cs], bf16, tag="vb", name=f"vb{m}")
                nc.scalar.copy(vb[:], vh[:])

                # fp32 scan output: the next chunk's s0 scalar (must be
                # fp32) reads the last column DIRECTLY -> the inter-chunk
                # carry is a same-engine dependency, no copy round-trip.
                # Both halves share one tile so the chunk needs ONE out-DMA.
                if m == 0:
                    co = cp.tile([128, 2 * cs], f32, tag="c", name="co")
                if c == 0:
                    init = cols[:, 2 * m:2 * m + 1]
                else:
                    pcs = csizes[c - 1]
                    init = c_hist[c - 1][:, m * pcs + pcs - 1:m * pcs + pcs]
                rep2 = [128, cs, 2]
                nc.vector._custom_dve(
                    mingru_op, out=co[:, m * cs:(m + 1) * cs],
                    in0=z[:].unsqueeze(2).broadcast_to(rep2),
                    in1=vb[:].unsqueeze(2).broadcast_to(rep2), s0=init)
            nc.sync.dma_start(
                out_d.rearrange("(m p) s -> p m s", m=2)[
                    :, :, c_start:c_start + cs],
                co[:].rearrange("p (m s) -> p m s", m=2))
            c_hist[c] = co
            c_start += cs

    nc.compile()
    return nc


def _get(seq_len, chunk):
    key = (seq_len, chunk)
    if key not in _CACHE:
        _CACHE[key] = _build(seq_len, chunk)
    return _CACHE[key]


def _make_in_maps(x, h0, w_h_w, w_h_b, w_z_w, w_z_b, n_cores=N_CORES):
    import ml_dtypes
    bf16 = ml_dtypes.bfloat16
    wzT = np.asarray(w_z_w, np.float32).T.astype(bf16)   # [D, H]
    whT = np.asarray(w_h_w, np.float32).T.astype(bf16)
    # packed weights [128, 4H]: [wz k0 | wz k1 | wh k0 | wh k1]
    wall = np.concatenate([wzT[:128], wzT[128:], whT[:128], whT[128:]],
                          axis=1)
    bz = np.asarray(w_z_b, np.float32).reshape(2, 128)
    bh = np.asarray(w_h_b, np.float32)
    in_maps = []
    for i in range(n_cores):
        c0 = (np.asarray(h0[i, 0], np.float32) - bh).reshape(2, 128)
        # [128, 4] = [c0_m0, bz_m0, c0_m1, bz_m1]
        cols = np.stack([c0[0], bz[0], c0[1], bz[1]], axis=1)
        xT = np.asarray(x[i], np.float32).T.astype(bf16)
        in_maps.append({
            "xT": np.ascontiguousarray(xT),
            "wall": np.ascontiguousarray(wall),
            "cols": np.ascontiguousarray(cols),
        })
    return in_maps


def kernel(x, h0, w_h_w, w_h_b, w_z_w, w_z_b):
    from concourse.bass_utils import run_bass_kernel_spmd

    nc = _get(S, 1024)
    in_maps = _make_in_maps(x, h0, w_h_w, w_h_b, w_z_w, w_z_b)
    res = run_bass_kernel_spmd(nc, in_maps, list(range(N_CORES)))
    bh = np.asarray(w_h_b, np.float32)
    out = np.stack(
        [np.asarray(res.results[i]["out"]).astype(np.float32).T + bh
         for i in range(N_CORES)], axis=0)
    return out.astype(np.float32)


# revision 48
# speedup vs baseline: 1.0106x; 1.0106x over previous
"""MinGRU Trainium2 kernel (nn_MinGRU_60421599920446).

Math (per batch row), with z = sigmoid(x@wz^T + bz), vh = x@wh^T:
    h_t = (1-z_t) h_{t-1} + z_t (vh_t + bh)
Substituting c_t = h_t - bh eliminates the bh bias from the device:
    c_t = (1-z_t) c_{t-1} + z_t vh_t,   c_0 = h0 - bh
The host adds bh back (and transposes) when assembling the output.

Strategy: data-parallel over batch, 1 row per NeuronCore (8 cores).
The host pre-transposes x to xT [D, S] in bf16, so the device does no
PE transposes and works natively in the scan layout [H partitions, S free].

The whole recurrence runs as ONE hand-written custom DVE op
(MINGRU_SCAN_ANT) that streams z (SBUF) and vh (PSUM fp32) directly:
    phase A (consumes z_t, v_t): u = 1-z; m = z*v; state *= u
    phase B (no consume):        state += m; emit state
Two uOp phases per timestep -> 2 cycles/step, same rate as the stock
tensor_tensor_scan, but with no a/b tensors, no PSUM->SBUF copies and
no Pool work at all.

Per 1024-step chunk (two H-halves m=0,1):
    sync DMA : xT[k] [128,1024] bf16 in
    PE       : vz[m], vh[m] [128,1024] fp32 PSUM   (16 matmuls)
    ACT      : z[m] = Sigmoid(vz[m]+bz) bf16
    DVE      : c[m] = MINGRU_SCAN_ANT(z[m], vh[m], carry) bf16
    sync DMA : c[m] -> cT [256, S] bf16 out
"""

import numpy as np
from contextlib import ExitStack

B, S, D, H = 8, 8192, 256, 256
N_CORES = 8

_CACHE = {}

OP_NAME = "MINGRU_SCAN_ANT"


def _register_mingru_op():
    """Register the hand-written fused scan as a custom DVE op (idempotent)."""
    import concourse.dve_ops as dve_ops
    for o in dve_ops.OPS:
        if o.name == OP_NAME:
            return o

    from concourse.dve_spec import Spec, Src0, Src1, C0
    from concourse.dve_uop import (
        DveOpSpec, UopConfig, UopDpConfig, AluOp, AluInp, InpSel, OutSel,
        OutPath, Trigger, DelayInp, ENABLE, N_STAGES,
    )

    def ref(in0, in1, c0, c1, c2):
        z = np.asarray(in0, np.float32)
        v = np.asarray(in1, np.float32)
        if z.ndim == 3:   # stride-0 repeat dim at the call site
            z = z[:, :, 0]
        if v.ndim == 3:
            v = v[:, :, 0]
        s = np.array(np.broadcast_to(c0, (z.shape[0], 1))[:, 0], np.float32)
        out = np.empty_like(z)
        for t in range(z.shape[1]):
            s = (1.0 - z[:, t]) * s + z[:, t] * v[:, t]
            out[:, t] = s
        return out

    spec = Spec(body=Src0 * Src1 + C0, reference=ref)  # body is a stand-in;
    # compile() below supplies the hand-written uOp program instead of lower().

    def build_uops(ver):
        n_stages = N_STAGES[ver]

        def mk():
            u = UopConfig()
            u.datapath_config = [UopDpConfig() for _ in range(n_stages)]
            return u

        # uop 0 -- seed: plant the carry (CONST_0, per-partition) in blk2's
        # out-flop. Consumes nothing, runs for exactly one slot.
        seed = mk()
        seed.enable_input(InpSel.SRC_0, 1)
        seed.enable_input(InpSel.SRC_1, 2)
        seed.enable_input(InpSel.CONST_0, 3)
        seed.datapath_config[0].pass_through_alu().pass_through_delay(2)
        seed.datapath_config[1].pass_through_alu().pass_through_delay(2)
        seed.datapath_config[2].enable_alu(AluOp.BYPASS, AluInp.PREV_DELAY_2)
        for k in range(3, n_stages):
            seed.datapath_config[k].pass_through_alu()
        seed.repeat_count = 1
        seed.trigger = (Trigger.COUNT, Trigger.NONE, Trigger.NONE)
        seed.next_uop = (1, 0, 0)

        # uop 1 -- phase A: consume (z_t, v_t). u = 1-z (blk0); m = z*v
        # (blk1, left in blk1's out-flop for phase B); state *= u (blk2,
        # via blk2's own CURR_ALU_OUT feedback).
        A = mk()
        A.enable_input(InpSel.SRC_0, 1)       # chain0 = z
        A.enable_input(InpSel.SRC_1, 2)       # chain1 = v
        A.enable_input(InpSel.ONE_F32, 3)     # chain2 = 1.0
        A.require_inp0 = ENABLE
        A.require_inp1 = ENABLE
        A.datapath_config[0].enable_alu(
            AluOp.SUBTRACT, AluInp.PREV_DELAY_2, AluInp.PREV_DELAY_0
        ).pass_through_delay(0, 1)
        A.datapath_config[1].enable_alu(
            AluOp.MULTIPLY, AluInp.PREV_DELAY_0, AluInp.PREV_DELAY_1
        ).enable_delay_from_src(DelayInp.PREV_ALU_OUT, 2)   # chain2 <- u
        A.datapath_config[2].enable_alu(
            AluOp.MULTIPLY, AluInp.CURR_ALU_OUT, AluInp.PREV_DELAY_2
        )
        for k in range(3, n_stages):
            A.datapath_config[k].pass_through_alu()
        A.repeat_count = 1
        # COUNT first: after consuming an element ALWAYS hand off to phase B
        # (even for the final element — exiting on SRC_TENSOR_DONE here would
        # skip the last emit and hang the dst-write count). SRC_TENSOR_DONE
        # only fires when A re-enters with the stream already exhausted.
        A.trigger = (Trigger.COUNT, Trigger.SRC_TENSOR_DONE, Trigger.NONE)
        A.next_uop = (2, 0, 0)

        # uop 2 -- phase B: no consume. blk1 re-presents m; blk2 adds it to
        # the state (CURR_ALU_OUT = u*state written by A one cycle earlier)
        # and the result rides to blk7 and is written out.
        # Each stream element is delivered TWICE (stride-0 inner AP dim at
        # the call site), so phase B is a real consuming element too — its
        # inputs are the same (z_t, v_t) again, ignored by the datapath.
        Bu = mk()
        Bu.enable_input(InpSel.SRC_0, 1)
        Bu.enable_input(InpSel.SRC_1, 2)
        Bu.require_inp0 = ENABLE
        Bu.require_inp1 = ENABLE
        Bu.datapath_config[1].enable_alu(AluOp.BYPASS, AluInp.CURR_ALU_OUT)
        Bu.datapath_config[2].enable_alu(
            AluOp.ADD, AluInp.CURR_ALU_OUT, AluInp.PREV_ALU_OUT
        )
        for k in range(3, n_stages):
            Bu.datapath_config[k].pass_through_alu()
        Bu.enable_output(OutSel.ALU_OUT, OutPath.WR0_LO)
        Bu.repeat_count = 1
        # B consumes the final stream element (index 2N-1): SRC_TENSOR_DONE
        # must fire HERE so the sequencer reaches IDLE and the next custom
        # instruction can dispatch (exiting only via a stalled A leaves the
        # FSM parked and hangs the following custom-DVE instruction).
        Bu.trigger = (Trigger.SRC_TENSOR_DONE, Trigger.COUNT, Trigger.NONE)
        Bu.next_uop = (0, 1, 0)

        for u in (seed, A, Bu):
            u.validate(ver)
        return [seed, A, Bu]

    class HandDveOp(dve_ops.DveOp):
        def compile(self, ver):
            key = (self.name, ver)
            cache = dve_ops._COMPILE_CACHE
            if key not in cache:
                cache[key] = DveOpSpec(
                    name=self.name,
                    opcode=dve_ops.get_dve_sub_opcode(self.name),
                    uops=build_uops(ver),
                    rd1_en=True,
                )
            return cache[key]

    op = HandDveOp(name=OP_NAME, spec=spec, subdim=False, uops_sha={})
    dve_ops.OPS.append(op)
    dve_ops.CUSTOM_DVE_SPECS[OP_NAME] = spec
    dve_ops._SUB_OPCODE_FOR_NAME[OP_NAME] = (
        dve_ops._CUSTOM_DVE_ROW_BASE + len(dve_ops.OPS) - 1
    )
    assert dve_ops._SUB_OPCODE_FOR_NAME[OP_NAME] < 0x20
    return op


OP2_NAME = "MINGRU_SCAN2_ANT"


def _register_mingru2_op():
    """Interleaved-halves fused scan: ONE instruction processes both H-halves
    at 1 timestep/cycle aggregate (2x the stock tensor_tensor_scan rate).

    Stream slots alternate halves (even slot = half0, odd = half1); the
    call site's 3D APs deliver (t, m)-interleaved elements from contiguous
    per-half tiles. Each slot consumes (z, v) of its half and performs the
    full update c = (1-z)c + z*v. Per-half state lives in a stage A-flop
    (half0: s4.A, half1: s5.A), read one stage earlier via NEXT_ALU_OUT_A:
    a 2-cycle feedback loop matching each half's 2-slot cadence.

    Carries enter THROUGH THE STREAM: the first (t=0) pair per half must be
    (z=1, v=carry), which sets state=carry exactly; its echoed output is
    sliced away by the caller. No seed uop, no scalar operands."""
    import concourse.dve_ops as dve_ops
    for o in dve_ops.OPS:
        if o.name == OP2_NAME:
            return o

    from concourse.dve_spec import Spec, Src0, Src1
    from concourse.dve_uop import (
        DveOpSpec, UopConfig, UopDpConfig, AluOp, AluInp, InpSel, OutSel,
        OutPath, Trigger, DelayInp, ENABLE, N_STAGES,
    )

    def ref(in0, in1, c0, c1, c2):
        z = np.asarray(in0, np.float32)
        v = np.asarray(in1, np.float32)
        flat = z.ndim == 2
        if flat:   # [P, 2T] memory-interleaved stream
            z = z.reshape(z.shape[0], -1, 2)
            v = v.reshape(v.shape[0], -1, 2)
        P, T, _ = z.shape
        st = [np.zeros(P, np.float32), np.zeros(P, np.float32)]
        out = np.empty_like(z)
        for t in range(T):
            for m in range(2):
                st[m] = (1.0 - z[:, t, m]) * st[m] + z[:, t, m] * v[:, t, m]
                out[:, t, m] = st[m]
        return out.reshape(P, -1) if flat else out

    spec = Spec(body=Src0 * Src1, reference=ref)  # stand-in body

    def build_uops(ver):
        n_stages = N_STAGES[ver]

        def steady(mult_stage, add_stage):
            U = UopConfig()
            U.datapath_config = [UopDpConfig() for _ in range(n_stages)]
            U.enable_input(InpSel.SRC_0, 1)       # chain0 = z
            U.enable_input(InpSel.SRC_1, 2)       # chain1 = v
            U.enable_input(InpSel.ONE_F32, 3)     # chain2 = 1.0
            U.require_inp0 = ENABLE
            U.require_inp1 = ENABLE
            # s0: u = 1 - z
            U.datapath_config[0].enable_alu(
                AluOp.SUBTRACT, AluInp.PREV_DELAY_2, AluInp.PREV_DELAY_0
            ).pass_through_delay(0, 1)
            # s1: m = z*v ; chain2 <- u
            U.datapath_config[1].enable_alu(
                AluOp.MULTIPLY, AluInp.PREV_DELAY_0, AluInp.PREV_DELAY_1
            ).enable_delay_from_src(DelayInp.PREV_ALU_OUT, 2)
            # s2: chain3 <- m ; carry chain2 (u)
            U.datapath_config[2].pass_through_alu().pass_through_delay(2)
            U.datapath_config[2].enable_delay_from_src(DelayInp.PREV_ALU_OUT, 3)
            for k in range(3, mult_stage):
                U.datapath_config[k].pass_through_alu().pass_through_delay(2, 3)
            # mult_stage: p = state * u  (state = A-flop of mult_stage+1)
            U.datapath_config[mult_stage].enable_alu(
                AluOp.MULTIPLY, AluInp.NEXT_ALU_OUT_A, AluInp.PREV_DELAY_2
            ).pass_through_delay(3)
            # add_stage: c' = p + m ; state <- c' (own A-flop)
            blk = U.datapath_config[add_stage]
            blk.enable_alu(AluOp.ADD, AluInp.PREV_ALU_OUT, AluInp.PREV_DELAY_3)
            blk.alu_out_a_enable = ENABLE
            for k in range(add_stage + 1, n_stages):
                U.datapath_config[k].pass_through_alu()
            U.enable_output(OutSel.ALU_OUT, OutPath.WR0_LO)
            U.repeat_count = 1
            return U

        # uops[0] cannot be looped back to, so the H0 config appears twice:
        # entry (index 0) and loop body (index 2).
        H0a = steady(3, 4)
        H0a.trigger = (Trigger.COUNT, Trigger.SRC_TENSOR_DONE, Trigger.NONE)
        H0a.next_uop = (1, 0, 0)
        H1 = steady(4, 5)
        H1.trigger = (Trigger.SRC_TENSOR_DONE, Trigger.COUNT, Trigger.NONE)
        H1.next_uop = (0, 2, 0)
        H0b = steady(3, 4)
        H0b.trigger = (Trigger.COUNT, Trigger.SRC_TENSOR_DONE, Trigger.NONE)
        H0b.next_uop = (1, 0, 0)

        for u in (H0a, H1, H0b):
            u.validate(ver)
        return [H0a, H1, H0b]

    class HandDveOp(dve_ops.DveOp):
        def compile(self, ver):
            key = (self.name, ver)
            cache = dve_ops._COMPILE_CACHE
            if key not in cache:
                cache[key] = DveOpSpec(
                    name=self.name,
                    opcode=dve_ops.get_dve_sub_opcode(self.name),
                    uops=build_uops(ver),
                    rd1_en=True,
                )
            return cache[key]

    op = HandDveOp(name=OP2_NAME, spec=spec, subdim=False, uops_sha={})
    dve_ops.OPS.append(op)
    dve_ops.CUSTOM_DVE_SPECS[OP2_NAME] = spec
    dve_ops._SUB_OPCODE_FOR_NAME[OP2_NAME] = (
        dve_ops._CUSTOM_DVE_ROW_BASE + len(dve_ops.OPS) - 1
    )
    assert dve_ops._SUB_OPCODE_FOR_NAME[OP2_NAME] < 0x20
    return op


def _build(seq_len, chunk):
    """Build + compile the single-core SPMD Bass program."""
    import concourse.bacc as bacc
    import concourse.tile as tile
    import concourse.mybir as mybir

    dt = mybir.dt
    f32 = dt.float32
    bf16 = dt.bfloat16
    AF = mybir.ActivationFunctionType

    mingru_op = _register_mingru_op()

    assert chunk % 512 == 0 and seq_len % chunk == 0
    if seq_len // chunk >= 4:
        # taper: small first/last chunks shorten pipeline fill (first scan
        # starts sooner) and drain (smaller final scans + output DMA)
        csizes = ([chunk // 2] + [chunk] * (seq_len // chunk - 1)
                  + [chunk // 2])
    else:
        csizes = [chunk] * (seq_len // chunk)
    assert sum(csizes) == seq_len

    nc = bacc.Bacc("TRN2", target_bir_lowering=False, debug=False)

    xT_d = nc.dram_tensor("xT", [D, seq_len], bf16, kind="ExternalInput").ap()
    # packed weights: [wz k0 | wz k1 | wh k0 | wh k1], each [128, H]
    wall_d = nc.dram_tensor("wall", [128, 4 * H], bf16, kind="ExternalInput").ap()
    # packed per-partition columns: [c0_m0, bz_m0, c0_m1, bz_m1]
    cols_d = nc.dram_tensor("cols", [128, 4], f32, kind="ExternalInput").ap()
    out_d = nc.dram_tensor("out", [H, seq_len], f32, kind="ExternalOutput").ap()

    with tile.TileContext(nc) as tc, ExitStack() as ctx:
        const = ctx.enter_context(tc.tile_pool(name="const", bufs=1))
        xin = ctx.enter_context(tc.tile_pool(name="xin", bufs=6))
        zp = ctx.enter_context(tc.tile_pool(name="z", bufs=6))
        vbp = ctx.enter_context(tc.tile_pool(name="vb", bufs=6))
        cp = ctx.enter_context(tc.tile_pool(name="c", bufs=4))
        vzp = ctx.enter_context(tc.tile_pool(name="vz", bufs=2, space="PSUM"))
        vhp = ctx.enter_context(tc.tile_pool(name="vh", bufs=2, space="PSUM"))

        cols = const.tile([128, 4], f32, tag="cols", name="cols")
        nc.sync.dma_start(cols[:], cols_d[:, :])
        wall = const.tile([128, 4 * H], bf16, tag="wall", name="wall")
        nc.sync.dma_start(wall[:], wall_d[:, :])
        # lhsT slice for matrix j (0=z, 1=h), k-half k, output half m
        def wsl(j, k, m):
            o = j * 2 * H + k * H + m * 128
            return wall[:, o:o + 128]

        c_hist = {}

        c_start = 0
        for c, cs in enumerate(csizes):
            mw = min(cs, 512)
            n512 = cs // mw
            xt = xin.tile([128, 2 * cs], bf16, tag="xt", name="xt")
            dma_eng = nc.sync
            dma_eng.dma_start(
                xt[:].rearrange("p (k s) -> p k s", k=2),
                xT_d.rearrange("(k p) s -> p k s", k=2)[
                    :, :, c_start:c_start + cs])

            for m in range(2):
                vz = vzp.tile([128, cs], f32, tag="vz", name=f"vz{m}")
                for k in range(2):
                    for s2 in range(n512):
                        nc.tensor.matmul(
                            vz[:, s2 * mw:(s2 + 1) * mw],
                            wsl(0, k, m),
                            xt[:, k * cs + s2 * mw:
                               k * cs + (s2 + 1) * mw],
                            start=(k == 0), stop=(k == 1),
                        )
                vh = vhp.tile([128, cs], f32, tag="vh", name=f"vh{m}")
                for k in range(2):
                    for s2 in range(n512):
                        nc.tensor.matmul(
                            vh[:, s2 * mw:(s2 + 1) * mw],
                            wsl(1, k, m),
                            xt[:, k * cs + s2 * mw:
                               k * cs + (s2 + 1) * mw],
                            start=(k == 0), stop=(k == 1),
                        )

                z = zp.tile([128, cs], bf16, tag="z", name=f"z{m}")
                nc.scalar.activation(z[:], vz[:], AF.Sigmoid,
                                     bias=cols[:, 2 * m + 1:2 * m + 2],
                                     scale=1.0)
                # custom DVE ops cannot read PSUM -> stage vh in SBUF (bf16)
                vb = vbp.tile([128, cs], bf16, tag="vb", name=f"vb{m}")
                nc.scalar.copy(vb[:], vh[:])

                # fp32 scan output: the next chunk's s0 scalar (must be
                # fp32) reads the last column DIRECTLY -> the inter-chunk
                # carry is a same-engine dependency, no copy round-trip.
                # Both halves share one tile so the chunk needs ONE out-DMA.
                if m == 0:
                    co = cp.tile([128, 2 * cs], f32, tag="c", name="co")
                if c == 0:
                    init = cols[:, 2 * m:2 * m + 1]
                else:
                    pcs = csizes[c - 1]
                    init = c_hist[c - 1][:, m * pcs + pcs - 1:m * pcs + pcs]
                rep2 = [128, cs, 2]
                nc.vector._custom_dve(
                    mingru_op, out=co[:, m * cs:(m + 1) * cs],
                    in0=z[:].unsqueeze(2).broadcast_to(rep2),
                    in1=vb[:].unsqueeze(2).broadcast_to(rep2), s0=init)
            nc.sync.dma_start(
                out_d.rearrange("(m p) s -> p m s", m=2)[
                    :, :, c_start:c_start + cs],
                co[:].rearrange("p (m s) -> p m s", m=2))
            c_hist[c] = co
            c_start += cs

    nc.compile()
    return nc


def _get(seq_len, chunk):
    key = (seq_len, chunk)
    if key not in _CACHE:
        _CACHE[key] = _build(seq_len, chunk)
    return _CACHE[key]


def _make_in_maps(x, h0, w_h_w, w_h_b, w_z_w, w_z_b, n_cores=N_CORES):
    import ml_dtypes
    bf16 = ml_dtypes.bfloat16
    wzT = np.asarray(w_z_w, np.float32).T.astype(bf16)   # [D, H]
    whT = np.asarray(w_h_w, np.float32).T.astype(bf16)
    # packed weights [128, 4H]: [wz k0 | wz k1 | wh k0 | wh k1]
    wall = np.concatenate([wzT[:128], wzT[128:], whT[:128], whT[128:]],
                          axis=1)
    bz = np.asarray(w_z_b, np.float32).reshape(2, 128)
    bh = np.asarray(w_h_b, np.float32)
    in_maps = []
    for i in range(n_cores):
        c0 = (np.asarray(h0[i, 0], np.float32) - bh).reshape(2, 128)
        # [128, 4] = [c0_m0, bz_m0, c0_m1, bz_m1]
        cols = np.stack([c0[0], bz[0], c0[1], bz[1]], axis=1)
        xT = np.asarray(x[i], np.float32).T.astype(bf16)
        in_maps.append({
            "xT": np.ascontiguousarray(xT),
            "wall": np.ascontiguousarray(wall),
            "cols": np.ascontiguousarray(cols),
        })
    return in_maps


def kernel(x, h0, w_h_w, w_h_b, w_z_w, w_z_b):
    from concourse.bass_utils import run_bass_kernel_spmd

    nc = _get(S, 1024)
    in_maps = _make_in_maps(x, h0, w_h_w, w_h_b, w_z_w, w_z_b)
    res = run_bass_kernel_spmd(nc, in_maps, list(range(N_CORES)))
    bh = np.asarray(w_h_b, np.float32)
    out = np.stack(
        [np.asarray(res.results[i]["out"]).astype(np.float32).T + bh
         for i in range(N_CORES)], axis=0)
    return out.astype(np.float32)
